# revision 30
# baseline (speedup 1.0000x reference)
"""Causal attention (B=4, L=4096, D=2048, HD=128) on 8 TRN2 NeuronCores.

Sharding: 8 cores = 4 batches x 2 fold-halves. Core c handles batch b=c//2
and q blocks {0,3} (i=c%2==0) or {1,2} (i==1), 1024 rows each — the fold
balances causal work. Each core computes K/V only for its OWN 2048 keys and
exchanges the other half with its pair-core via two 2-rank AllGathers
(0.5 MB each) that overlap projection compute, instead of recomputing.

Local K/V column map (kt_s / v_s, 5x1024):
  A [0:1024)    own block 0   (g0 for i=0, g1 for i=1)
  B [1024:2048) own block 1   (g3 / g2)
  C [2048:3072) AG1 row 0 = g0
  D [3072:4096) AG1 row 1 = g1
  F [4096:5120) AG2 row 1 = g2
The AllGather output rows are rank-ordered, hence absolute (g-indexed), so
one SPMD program works for both fold halves; per-core behavior comes from
the data: slot-bias vectors enable/disable the two fold-dependent slots.

Attention slots (per 512-q unit):  phase A (lq0): A diag, C gated "bA"
(i=1 on);  phase B (lq1): C full, B diag, D full, F gated "bB" (i=0 on).

Matmuls run bf16; the post-softmax AV stage consumes the exp output cast
to bf16 on the ACT engine. Out-projection is written unnormalized as bf16
with a separate f32 rowsum vector; the host divides and adds bo.

Layouts (partition dim first):
  xT      [D=2048, 2048]  x[b].T own columns
  Qt, Kt  [HD=128, n]     projections, head dim on partitions
  V       [k%128, 32*128] transposed per 128-key tile: [k, hd]
  scores  [k=128, q=512]  one matmul per tile; exp+slot-bias on ACT
  outT    [D, q] bf16     final projection, transposed; host normalizes
"""

import numpy as np
import ml_dtypes

B, L, D, HD = 4, 4096, 2048, 128
BLK = 1024            # fold block (4 per batch)
LQ = 2 * BLK          # queries per core
LKL = 5 * BLK         # local key columns (A,B,C,D,F)
ND = D // 128         # 16 d-tiles
NRB = LQ // 512       # 4 own column blocks
NEG = -50.0           # slot-disable bias (exp(x-50) ~ 0)
MASKVAL = -30000.0    # intra-tile causal mask additive value
RG = [[0, 1], [2, 3], [4, 5], [6, 7]]

_cached = {}


def _build_program():
    import concourse.bass as bass
    import concourse.tile as tile
    from concourse import bacc, mybir
    from concourse.masks import make_identity

    f32 = mybir.dt.float32
    bf16 = mybir.dt.bfloat16
    nc = bacc.Bacc("TRN2", target_bir_lowering=False, debug=False,
                   num_devices=8)

    xT_d = nc.dram_tensor("xT", (D, LQ), bf16, kind="ExternalInput")
    wq_d = nc.dram_tensor("wq", (128 * ND, HD), bf16, kind="ExternalInput")
    wk_d = nc.dram_tensor("wk", (128 * ND, HD), bf16, kind="ExternalInput")
    wv_d = nc.dram_tensor("wv", (128 * ND, HD), bf16, kind="ExternalInput")
    wo_d = nc.dram_tensor("wo", (HD, D), bf16, kind="ExternalInput")
    bias_d = nc.dram_tensor("biases", (128, 8), f32, kind="ExternalInput")
    out_d = nc.dram_tensor("outT", (D, LQ), bf16, kind="ExternalOutput")
    rs_d = nc.dram_tensor("rowsums", (1, LQ), f32, kind="ExternalOutput")

    # phase -> list of (local_kblk, kind); kind in {"diag", "full", "bA", "bB"}
    SLOTS = {
        0: [(0, "diag"), (2, "bA")],
        1: [(2, "full"), (1, "diag"), (3, "full"), (4, "bB")],
    }

    with tile.TileContext(nc) as tc:
        with (
            tc.tile_pool(name="const", bufs=1) as cpool,
            tc.tile_pool(name="xt", bufs=4) as xtpool,
            tc.tile_pool(name="vt", bufs=3) as vtpool,
            tc.tile_pool(name="expst", bufs=12) as epool,
            tc.tile_pool(name="outsb", bufs=6) as outpool,
            tc.tile_pool(name="dram", bufs=1, space="DRAM") as dpool,
            tc.tile_pool(name="psum", bufs=1, space="PSUM") as psum,
        ):
            # ---- persistent SBUF tensors ----
            wq_s = cpool.tile([128, ND, 128], bf16, tag="wq")
            wk_s = cpool.tile([128, ND, 128], bf16, tag="wk")
            wv_s = cpool.tile([128, ND, 128], bf16, tag="wv")
            wo_s = cpool.tile([128, D], bf16, tag="wo")
            bias_s = cpool.tile([128, 8], f32, tag="biases")
            kt_s = cpool.tile([128, LKL], bf16, tag="kt")
            qt_s = cpool.tile([128, LQ], bf16, tag="qt")
            v_s = cpool.tile([128, LKL], bf16, tag="v")
            ones_s = cpool.tile([128, 1], bf16, tag="ones")
            rs_s = cpool.tile([1, LQ], f32, tag="rs")
            masks_s = cpool.tile([128, 4 * 512], f32, tag="masks")
            ot_s = cpool.tile([128, LQ], bf16, tag="ot")
            identb_s = cpool.tile([128, 128], bf16, tag="identb")

            # collective bounce buffers (DRAM)
            cc_in = [dpool.tile([128, 2048], bf16, tag=f"ccin{j}",
                                name=f"ccin{j}") for j in range(2)]
            cc_out = [dpool.tile([256, 2048], bf16, tag=f"ccout{j}",
                                 name=f"ccout{j}") for j in range(2)]


            xT_r = xT_d.ap().rearrange("(n p) m -> p n m", p=128)
            wq_r = wq_d.ap().rearrange("(p n) m -> p n m", n=ND)
            wk_r = wk_d.ap().rearrange("(p n) m -> p n m", n=ND)
            wv_r = wv_d.ap().rearrange("(p n) m -> p n m", n=ND)

            # Each DMA trigger costs ~650ns serialized on the sync queue:
            # wk + xt0 first (gate the first matmul), xt1 next (avoids the
            # rb1 stall), the rest after. Weights are single triggers.
            def xt_fetch(rb):
                xts[rb] = xtpool.tile([128, ND, 512], bf16, tag="xt",
                                      name="xt")
                for ch in range(4):
                    nc.sync.dma_start(
                        xts[rb][:, ch * 4:(ch + 1) * 4, :],
                        xT_r[:, ch * 4:(ch + 1) * 4,
                             rb * 512:(rb + 1) * 512],
                    )

            xts = {}
            nc.sync.dma_start(wk_s[:], wk_r[:])
            xt_fetch(0)
            xt_fetch(1)
            nc.sync.dma_start(wv_s[:], wv_r[:])
            nc.sync.dma_start(wq_s[:], wq_r[:])
            nc.sync.dma_start(wo_s[:], wo_d.ap())
            nc.sync.dma_start(bias_s[:], bias_d.ap())

            make_identity(nc, identb_s[:])
            nc.gpsimd.memset(ones_s[:], 1.0)

            # 4 causal mask tiles for relative offsets delta = 0,128,256,384:
            # keep 0 where q_free >= k_part + delta, else MASKVAL
            nc.gpsimd.memset(masks_s[:], 0.0)
            for m in range(4):
                nc.gpsimd.affine_select(
                    out=masks_s[:, m * 512:(m + 1) * 512],
                    in_=masks_s[:, m * 512:(m + 1) * 512],
                    compare_op=mybir.AluOpType.is_ge,
                    fill=MASKVAL,
                    base=-(m * 128),
                    channel_multiplier=-1,
                    pattern=[[1, 512]],
                )

            bq_ap = bias_s[:, 0:1]
            bk_ap = bias_s[:, 1:2]
            bv_ap = bias_s[:, 2:3]
            slot_bias = {"bA": bias_s[:, 3:4], "bB": bias_s[:, 4:5]}

            def emit_rb(rb, prefetch_rb=None):
                """K/V projections for one 512-wide own column block.

                Q is deferred (emit_q) so K/V — the collective inputs —
                finish as early as possible; Q fills the AG1 wait."""
                xt = xts[rb]
                if prefetch_rb is not None:
                    xt_fetch(prefetch_rb)
                cs = slice(rb * 512, (rb + 1) * 512)

                pk = psum.tile([128, 512], f32, tag="acc512", bufs=2, name="pk")
                for dt in range(ND):
                    nc.tensor.matmul(
                        pk[:], wk_s[:, dt, :], xt[:, dt, :],
                        start=(dt == 0), stop=(dt == ND - 1),
                    )
                nc.vector.tensor_scalar_add(kt_s[:, cs], pk[:], bk_ap)

                pv = psum.tile([128, 512], f32, tag="acc512", bufs=2, name="pv")
                for dt in range(ND):
                    nc.tensor.matmul(
                        pv[:], wv_s[:, dt, :], xt[:, dt, :],
                        start=(dt == 0), stop=(dt == ND - 1),
                    )
                vt_tmp = vtpool.tile([128, 512], bf16, tag="vt_tmp")
                nc.vector.tensor_scalar_add(vt_tmp[:], pv[:], bv_ap)
                for s in range(4):
                    ktile = rb * 4 + s
                    vp = psum.tile([128, 128], bf16, tag="acc512", bufs=2,
                                   name="vp")
                    nc.tensor.transpose(
                        vp[:], vt_tmp[:, s * 128:(s + 1) * 128], identb_s[:]
                    )
                    nc.vector.tensor_copy(
                        v_s[:, ktile * 128:(ktile + 1) * 128], vp[:]
                    )

            def emit_q(rb):
                xt = xts.pop(rb)
                cs = slice(rb * 512, (rb + 1) * 512)
                pq = psum.tile([128, 512], f32, tag="acc512", bufs=2, name="pq")
                for dt in range(ND):
                    nc.tensor.matmul(
                        pq[:], wq_s[:, dt, :], xt[:, dt, :],
                        start=(dt == 0), stop=(dt == ND - 1),
                    )
                nc.vector.tensor_scalar_add(qt_s[:, cs], pq[:], bq_ap)

            def emit_exchange(j):
                """Bounce own K/V block j (1024 cols) and AllGather it.

                AG output rows are rank-ordered: row half 0 = fold-core 0's
                block j, half 1 = fold-core 1's. Back-DMA into C/D (j=0)
                and F (j=1)."""
                cs = slice(j * 1024, (j + 1) * 1024)
                for h in range(2):
                    hs = slice(j * 1024 + h * 512, j * 1024 + (h + 1) * 512)
                    ds = slice(h * 512, (h + 1) * 512)
                    nc.sync.dma_start(cc_in[j][:, ds], kt_s[:, hs])
                    nc.sync.dma_start(cc_in[j][:, 1024 + h * 512:
                                               1024 + (h + 1) * 512],
                                      v_s[:, hs])
                nc.gpsimd.collective_compute(
                    "AllGather", mybir.AluOpType.bypass, replica_groups=RG,
                    ins=[cc_in[j][:]], outs=[cc_out[j][:]],
                )
                # rows 0:128 = g-lower block, 128:256 = g-upper block
                if j == 0:
                    dsts = [(0, 2048), (1, 3072)]   # C=g0, D=g1
                else:
                    dsts = [(1, 4096)]              # F=g2
                for half, base in dsts:
                    r = slice(half * 128, (half + 1) * 128)
                    nc.sync.dma_start(kt_s[:, base:base + 1024],
                                      cc_out[j][r, 0:1024])
                    nc.sync.dma_start(v_s[:, base:base + 1024],
                                      cc_out[j][r, 1024:2048])

            def emit_attn_u(phase, u, filler=None):
                q0 = phase * BLK + u * 512
                klist = []  # (ktile_global, mask_idx or None, bias_key)
                for kblk, kind in SLOTS[phase]:
                    for t in range(8):
                        if kind == "diag":
                            drel = t * 128 - u * 512
                            if drel >= 512:
                                continue
                            midx = drel // 128 if drel >= 0 else None
                            klist.append((kblk * 8 + t, midx, None))
                        else:
                            bkey = kind if kind in slot_bias else None
                            klist.append((kblk * 8 + t, None, bkey))

                ot_acc = psum.tile([128, 512], f32, tag="otacc", bufs=1,
                                   name="ot_acc")
                rs_acc = psum.tile([1, 512], f32, tag="rs", bufs=1,
                                   name="rs_acc")
                # k-tiles processed in pairs: two 512-score matmuls land in
                # the two banks of one wide PSUM tile, then ONE 1024-wide
                # exp on ACT (amortizes the ~350-cycle ACTIVATE overhead).
                n = len(klist)
                assert n % 2 == 0
                np_ = n // 2
                pairs = [(klist[2 * j_], klist[2 * j_ + 1])
                         for j_ in range(np_)]
                ests = [None] * np_

                def emit_st(pi_):
                    (kta, midxa, bkeya), (ktb, midxb, bkeyb) = pairs[pi_]
                    assert bkeya == bkeyb
                    st = psum.tile([128, 1024], f32, tag="st", bufs=2,
                                   name="st")
                    for h, kt in ((0, kta), (1, ktb)):
                        nc.tensor.matmul(
                            st[:, h * 512:(h + 1) * 512],
                            kt_s[:, kt * 128:(kt + 1) * 128],
                            qt_s[:, q0:q0 + 512],
                            start=True, stop=True,
                        )
                    for h, midx in ((0, midxa), (1, midxb)):
                        if midx is not None:
                            nc.vector.tensor_add(
                                st[:, h * 512:(h + 1) * 512],
                                st[:, h * 512:(h + 1) * 512],
                                masks_s[:, midx * 512:(midx + 1) * 512],
                            )
                    est = epool.tile([128, 1024], bf16, tag="est", bufs=6)
                    nc.scalar.activation(
                        est[:], st[:],
                        mybir.ActivationFunctionType.Exp,
                        bias=slot_bias[bkeya] if bkeya else 0.0,
                    )
                    ests[pi_] = est

                ngroups = (np_ + 1) // 2
                group_hs = []
                gi = 0
                emit_st(0)
                if np_ > 1:
                    emit_st(1)
                for pi_ in range(np_):
                    (kta, _, _), (ktb, _, _) = pairs[pi_]
                    if pi_ + 2 < np_:
                        emit_st(pi_ + 2)
                    if filler is not None:
                        next(filler, None)
                    est = ests[pi_]
                    for h, kt in ((0, kta), (1, ktb)):
                        nc.tensor.matmul(
                            ot_acc[:],
                            v_s[:, kt * 128:(kt + 1) * 128],
                            est[:, h * 512:(h + 1) * 512],
                            start=(pi_ == 0 and h == 0),
                            stop=(pi_ == np_ - 1 and h == 1),
                        )
                    # row-sums: halves-add each wide est on gpsimd, pair
                    # the results on DVE, one rowsum matmul per 4 k-tiles
                    hs = epool.tile([128, 512], bf16, tag="esum", bufs=4,
                                    name="hs")
                    nc.gpsimd.tensor_add(hs[:], est[:, 0:512],
                                         est[:, 512:1024])
                    group_hs.append(hs)
                    if len(group_hs) == 2 or pi_ == np_ - 1:
                        g = group_hs
                        if len(g) == 1:
                            rs_rhs = g[0]
                        else:
                            esum = epool.tile([128, 512], bf16, bufs=4,
                                              tag="esum3", name="esum")
                            nc.vector.tensor_add(esum[:], g[0][:], g[1][:])
                            rs_rhs = esum
                        nc.tensor.matmul(
                            rs_acc[:], ones_s[:], rs_rhs[:],
                            start=(gi == 0), stop=(gi == ngroups - 1),
                        )
                        group_hs = []
                        gi += 1

                qb = phase * 2 + u
                nc.vector.tensor_copy(
                    ot_s[:, qb * 512:(qb + 1) * 512], ot_acc[:]
                )
                nc.vector.tensor_copy(
                    rs_s[:, qb * 512:(qb + 1) * 512], rs_acc[:]
                )

            out_r = out_d.ap().rearrange("(n p) m -> p n m", p=128)

            def outproj_filler(qb, dts):
                for dt in dts:
                    emit_outproj(qb, [dt])
                    yield

            def emit_outproj(qb, dts):
                # out-projection chunk (unnormalized; host divides by
                # rowsums). Copies on DVE; DMAs batched 4 d-tiles per
                # trigger (each trigger costs ~650ns on the sync queue).
                for dt in dts:
                    po = psum.tile([128, 512], f32, tag="acc512", bufs=2,
                                   name="po")
                    nc.tensor.matmul(
                        po[:],
                        wo_s[:, dt * 128:(dt + 1) * 128],
                        ot_s[:, qb * 512:(qb + 1) * 512],
                        start=True, stop=True,
                    )
                    if dt % 4 == 0:
                        orows[qb] = outpool.tile([128, 4, 512], bf16,
                                                 tag="orow", bufs=4,
                                                 name="orow")
                    nc.vector.tensor_copy(orows[qb][:, dt % 4, :], po[:])
                    if dt % 4 == 3:
                        nc.sync.dma_start(
                            out_r[:, dt - 3:dt + 1,
                                  qb * 512:(qb + 1) * 512],
                            orows[qb][:],
                        )

            def drain(gen):
                if gen is not None:
                    for _ in gen:
                        pass

            orows = {}

            # ---- schedule ----
            # K/V (collective inputs) first, Q deferred to fill the AG1
            # wait; attention phases ACT-paced with outproj fillers.
            emit_rb(0, prefetch_rb=2)
            emit_rb(1, prefetch_rb=3)
            emit_exchange(0)
            emit_rb(2)
            emit_rb(3)
            emit_exchange(1)
            for rb in range(NRB):
                emit_q(rb)
            emit_attn_u(0, 0)
            f = outproj_filler(0, range(ND))
            emit_attn_u(0, 1, filler=f)
            drain(f)
            f = outproj_filler(1, range(ND))
            emit_attn_u(1, 0, filler=f)
            drain(f)
            f = outproj_filler(2, range(ND))
            emit_attn_u(1, 1, filler=f)
            drain(f)
            emit_outproj(3, range(ND))
            nc.sync.dma_start(rs_d.ap(), rs_s[:])

    nc.compile()
    return nc


def _get_program():
    if "nc" not in _cached:
        _cached["nc"] = _build_program()
    return _cached["nc"]


def _own_blocks(i):
    return [0, 3] if i == 0 else [1, 2]


def _pack_w(W):
    # (D, HD) -> [p=128, nd=16, m] contiguous -> (128*16, HD)
    return np.ascontiguousarray(
        W.astype(ml_dtypes.bfloat16).reshape(ND, 128, HD).transpose(1, 0, 2)
    ).reshape(128 * ND, HD)


def make_in_maps(x, Wq, bq, Wk, bk, Wv, bv, Wo, bo):
    scale = 1.0 / np.sqrt(np.float32(HD))
    wq_s = (Wq * scale).astype(np.float32)
    bq_s = (bq * scale).astype(np.float32)
    wq_p = _pack_w(wq_s)
    wk_p = _pack_w(Wk)
    wv_p = _pack_w(Wv)
    wo_b = Wo.astype(ml_dtypes.bfloat16)
    in_maps = []
    for c in range(8):
        i, b = c % 2, c // 2
        own = _own_blocks(i)
        xbT = x[b].T  # (D, L) view
        xT = np.concatenate(
            [xbT[:, p * BLK:(p + 1) * BLK] for p in own], axis=1
        ).astype(ml_dtypes.bfloat16)
        biases = np.zeros((128, 8), np.float32)
        biases[:, 0] = bq_s
        biases[:, 1] = bk.astype(np.float32)
        biases[:, 2] = bv.astype(np.float32)
        biases[:, 3] = NEG if i == 0 else 0.0   # phase A slot C ("bA")
        biases[:, 4] = 0.0 if i == 0 else NEG   # phase B slot F ("bB")
        in_maps.append({
            "xT": np.ascontiguousarray(xT),
            "wq": wq_p,
            "wk": wk_p,
            "wv": wv_p,
            "wo": wo_b,
            "biases": biases,
        })
    return in_maps


def assemble_output(results, bo):
    out = np.empty((B, L, D), np.float32)
    for c in range(8):
        i, b = c % 2, c // 2
        own = _own_blocks(i)
        outT = (results[c]["outT"].astype(np.float32)
                / results[c]["rowsums"])  # (D, LQ)
        qA, qB = own[0], own[1]
        out[b, qA * BLK:(qA + 1) * BLK, :] = outT[:, 0:BLK].T
        out[b, qB * BLK:(qB + 1) * BLK, :] = outT[:, BLK:2 * BLK].T
    out += bo.astype(np.float32)
    return out


def kernel(x, Wq, bq, Wk, bk, Wv, bv, Wo, bo):
    from concourse.bass_utils import run_bass_kernel_spmd

    nc = _get_program()
    in_maps = make_in_maps(
        np.asarray(x), np.asarray(Wq), np.asarray(bq), np.asarray(Wk),
        np.asarray(bk), np.asarray(Wv), np.asarray(bv), np.asarray(Wo),
        np.asarray(bo),
    )
    res = run_bass_kernel_spmd(nc, in_maps, core_ids=list(range(8)))
    return assemble_output(res.results, np.asarray(bo))
